# revision 10
# baseline (speedup 1.0000x reference)
"""Walsh-Hadamard transform (Sylvester order) along rows of a [16384, 4096]
fp32 matrix, on 8 Trainium2 NeuronCores.

Algorithm: H_4096 = H_32 (kron) H_128 with i = i_hi*128 + i_lo. Per row x,
reshaped to X[j_hi(32), j_lo(128)]:  Y = H32 @ X @ H128  (both symmetric).
On the PE this is a chain  T -> (H128 @ .) -> T -> (blockdiag H32 @ .)
over 128x128 blocks, with 4 batch rows packed per partition group so every
matmul contracts a full K=128.

Sharding: batch dim / 8 cores (2048 rows per core), no communication.

Layouts (per core, b4 = row//4, g = row%4):
  load  L[p=(g,jh), f=(b4, jl)] = x[4*b4+g, jh*128+jl]   (512B DRAM runs)
  T1 per-b4 128x128 transpose    -> [p=jl, f=(b4, g,jh)]
  MM1 lhsT=H128 contract jl      -> [p=il, f=(b4, g,jh)]
  T2 per-b4 128x128 transpose    -> [p=(g,jh), f=(b4, il)]
  MM2 lhsT=blockdiag(H32 x4)     -> [p=(g,ih), f=(b4, il)]
  store y[4*b4+g, ih*128+il]                              (512B DRAM runs)
"""

import os
import sys

import numpy as np

if "/opt/trn_rl_repo" not in sys.path:
    sys.path.insert(0, "/opt/trn_rl_repo")

NCORES = 8
BATCH = 16384
N = 4096
ROWS = BATCH // NCORES  # 2048 rows per core
NB4 = ROWS // 4  # 512 groups of 4 rows

# --- tunables ---------------------------------------------------------------
SUPER_B4 = int(os.environ.get("WHT_SUPER_B4", "16"))  # b4/DMA supertile
CHUNK_B4 = int(os.environ.get("WHT_CHUNK_B4", "4"))  # b4/compute chunk
PS_BUFS = int(os.environ.get("WHT_PS_BUFS", "0"))  # 0 = auto (8 banks)
SB_BUFS = int(os.environ.get("WHT_SB_BUFS", "4"))
MM_DTYPE = os.environ.get("WHT_MM_DTYPE", "f32r")  # "f32r" | "f32"
REPEAT = int(os.environ.get("WHT_REPEAT", "1"))  # timing amplification
# ----------------------------------------------------------------------------


def _sylvester(k: int) -> np.ndarray:
    H = np.array([[1.0]], dtype=np.float64)
    for _ in range(k):
        H = np.block([[H, H], [H, -H]])
    return H


def _consts():
    h128 = _sylvester(7).astype(np.float32)
    hbd32 = np.kron(np.eye(4), _sylvester(5)).astype(np.float32)
    ident = np.eye(128, dtype=np.float32)
    return {"h128": h128, "hbd32": hbd32, "ident": ident}


_NC_CACHE = {}


def build_nc():
    key = (SUPER_B4, CHUNK_B4, MM_DTYPE, PS_BUFS, SB_BUFS, REPEAT)
    if key in _NC_CACHE:
        return _NC_CACHE[key]

    import concourse.tile as tile
    from concourse import bacc, mybir

    f32 = mybir.dt.float32
    f32r = mybir.dt.float32r

    nc = bacc.Bacc(
        "TRN2",
        target_bir_lowering=False,
        debug=False,
        num_devices=NCORES,
    )
    xs = nc.dram_tensor("xs", [ROWS, N], f32, kind="ExternalInput").ap()
    ys = nc.dram_tensor("ys", [ROWS, N], f32, kind="ExternalOutput").ap()
    h128_d = nc.dram_tensor("h128", [128, 128], f32, kind="ExternalInput").ap()
    hbd_d = nc.dram_tensor("hbd32", [128, 128], f32, kind="ExternalInput").ap()
    id_d = nc.dram_tensor("ident", [128, 128], f32, kind="ExternalInput").ap()

    # flat element offset = b4*16384 + p*128 + jl  (p = g*32 + jh)
    x_re = xs.flatten().rearrange("(b4 p jl) -> p b4 jl", p=128, jl=128)
    y_re = ys.flatten().rearrange("(b4 p il) -> p b4 il", p=128, il=128)

    n_super = NB4 // SUPER_B4
    FW = SUPER_B4 * 128
    CW = CHUNK_B4 * 128
    n_chunk = SUPER_B4 // CHUNK_B4

    # FP32r matmul inputs must be *rounded to f32r by their producer*
    # (walrus birverifier rule), so the SBUF tiles feeding matmuls are
    # allocated as float32r and the copies into them perform the rounding.
    mm_dt = f32r if MM_DTYPE == "f32r" else f32

    # PSUM: 8 banks of 512 fp32 columns. 4 stage pools must fit.
    banks_per_tile = CW // 512
    ps_bufs = PS_BUFS if PS_BUFS else max(1, 8 // (4 * banks_per_tile))

    with tile.TileContext(nc) as tc:
        with (
            tc.tile_pool(name="consts", bufs=1) as cpool,
            tc.tile_pool(name="load", bufs=3) as lpool,
            tc.tile_pool(name="store", bufs=3) as spool,
            tc.tile_pool(name="st1", bufs=SB_BUFS) as p_st1,
            tc.tile_pool(name="sm1", bufs=SB_BUFS) as p_sm1,
            tc.tile_pool(name="st2", bufs=SB_BUFS) as p_st2,
            tc.tile_pool(name="ps_t1", bufs=ps_bufs, space="PSUM") as ps_t1,
            tc.tile_pool(name="ps_m1", bufs=ps_bufs, space="PSUM") as ps_m1,
            tc.tile_pool(name="ps_t2", bufs=ps_bufs, space="PSUM") as ps_t2,
            tc.tile_pool(name="ps_m2", bufs=ps_bufs, space="PSUM") as ps_m2,
        ):
            c_h128_raw = cpool.tile([128, 128], f32)
            nc.sync.dma_start(c_h128_raw[:], h128_d)
            c_hbd_raw = cpool.tile([128, 128], f32)
            nc.sync.dma_start(c_hbd_raw[:], hbd_d)
            c_id = cpool.tile([128, 128], f32)
            nc.sync.dma_start(c_id[:], id_d)
            if mm_dt is f32r:
                c_h128 = cpool.tile([128, 128], f32r, tag="h128r")
                nc.vector.tensor_copy(c_h128[:], c_h128_raw[:])
                c_hbd = cpool.tile([128, 128], f32r, tag="hbdr")
                nc.vector.tensor_copy(c_hbd[:], c_hbd_raw[:])
            else:
                c_h128, c_hbd = c_h128_raw, c_hbd_raw

            def mm_split(out_ps, lhsT, rhs):
                # fp32 moving operand caps at N=512 (one PSUM bank)
                for h in range(0, CW, 512):
                    w = min(512, CW - h)
                    nc.tensor.matmul(
                        out_ps[:, h : h + w], lhsT, rhs[:, h : h + w],
                        start=True, stop=True,
                    )

            for _rep in range(REPEAT):
                for s in range(n_super):
                    L = lpool.tile([128, FW], f32)
                    nc.sync.dma_start(
                        L[:].rearrange("p (b f) -> p b f", f=128),
                        x_re[:, s * SUPER_B4 : (s + 1) * SUPER_B4, :],
                    )
                    S = spool.tile([128, FW], f32)
                    for c in range(n_chunk):
                        pt1 = ps_t1.tile([128, CW], f32)
                        for q in range(CHUNK_B4):
                            col = (c * CHUNK_B4 + q) * 128
                            nc.tensor.transpose(
                                pt1[:, q * 128 : (q + 1) * 128],
                                L[:, col : col + 128],
                                c_id[:],
                            )
                        st1 = p_st1.tile([128, CW], mm_dt)
                        nc.vector.tensor_copy(st1[:], pt1[:])

                        pm1 = ps_m1.tile([128, CW], f32)
                        mm_split(pm1, c_h128[:], st1[:])
                        sm1 = p_sm1.tile([128, CW], f32)
                        nc.scalar.copy(sm1[:], pm1[:])

                        pt2 = ps_t2.tile([128, CW], f32)
                        for q in range(CHUNK_B4):
                            nc.tensor.transpose(
                                pt2[:, q * 128 : (q + 1) * 128],
                                sm1[:, q * 128 : (q + 1) * 128],
                                c_id[:],
                            )
                        st2 = p_st2.tile([128, CW], mm_dt)
                        nc.vector.tensor_copy(st2[:], pt2[:])

                        pm2 = ps_m2.tile([128, CW], f32)
                        mm_split(pm2, c_hbd[:], st2[:])
                        nc.scalar.copy(S[:, c * CW : (c + 1) * CW], pm2[:])

                    nc.sync.dma_start(
                        y_re[:, s * SUPER_B4 : (s + 1) * SUPER_B4, :],
                        S[:].rearrange("p (b f) -> p b f", f=128),
                    )

    nc.finalize()  # Bacc.finalize runs compile() (register alloc, DCE) first
    _NC_CACHE[key] = nc
    return nc


def _run(x: np.ndarray, trace: bool = False):
    from concourse.bass_utils import run_bass_kernel_spmd

    nc = build_nc()
    consts = _consts()
    x = np.ascontiguousarray(np.asarray(x, dtype=np.float32))
    in_maps = [
        dict(xs=x[i * ROWS : (i + 1) * ROWS], **consts) for i in range(NCORES)
    ]
    res = run_bass_kernel_spmd(nc, in_maps, list(range(NCORES)), trace=trace)
    y = np.concatenate([r["ys"] for r in res.results], axis=0)
    return y, res


def kernel(x, n=None, **kwargs) -> np.ndarray:
    y, _ = _run(x, trace=False)
    return y


if __name__ == "__main__":
    rng = np.random.default_rng(0)
    x = rng.standard_normal((BATCH, N)).astype(np.float32)
    y = kernel(x, N)
    print("out shape:", y.shape, y.dtype)
